# revision 35
# baseline (speedup 1.0000x reference)
"""Trainium2 Bass kernel for causal multi-head attention with RoPE.

Problem: x[2,2048,2048] -> qkv proj -> RoPE(q,k) -> causal softmax attention
(16 heads, hd=128) -> out proj.  Sharding: tensor-parallel over heads
(2 heads/core x 8 cores); the output projection contraction is restored
with one AllToAll per batch (head-shards -> sequence-shards), overlapped
with the other batch's compute, so each core computes a disjoint
[2, 256, 2048] slice of the final output.

Schedule highlights:
- startup DMAs split across sync/gpsimd/scalar queues in first-use order
  (k-split x chunk + per-head weights) so the PE starts ~25us earlier.
- RoPE applied per (head-block, s-chunk) right after each PSUM->SBUF
  copy, k-heads first, so attention never waits on a bulk RoPE pass.
- softmax denominators accumulated on the PE itself: per (key-tile,
  chunk) a [128,4] ones-column matmul adds exp-tile partial sums into
  row j of a dedicated PSUM bank (no vector/gpsimd accumulate chain).
- attention out + w_out are cast to bf16: AllToAll payload halves, and
  all four w_out column chunks fit in two resident 32KB SBUF slots, so
  batch-0 out-proj runs contiguously while the last AllToAll completes,
  then batch-1 with zero refetch.
"""

import os
import sys

if "/opt/trn_rl_repo" not in sys.path:
    sys.path.insert(0, "/opt/trn_rl_repo")

import numpy as np
import ml_dtypes

B, S, D = 2, 2048, 2048
H, HD = 16, 128
NCORES = 8
HPC = H // NCORES          # heads per core (2)
ROPE_BASE = 10000.0
SCALE = 1.0 / float(np.sqrt(HD))
SC = 512                   # QKV matmul free-dim chunk (s positions)
KSUB = D // 128            # 16 contraction subtiles
SCW = S // NCORES          # 256: per-core output cols per batch
MORDER = (2, 0, 3, 1)      # k-head blocks first (attention needs them first)

_CACHE = {}


def _install_trace_shim():
    """Optionally register the axon NTFF profile hook (for test.py tracing)."""
    try:
        import types

        if "antenv.axon_hooks" in sys.modules:
            return True
        import antenv
        from trn_agent_boot.trn_boot import _ntff_profile_via_ctypes

        hook = _ntff_profile_via_ctypes("/opt/axon/libaxon_pjrt.so")
        mod = types.ModuleType("antenv.axon_hooks")
        _state = {"hook": hook}
        mod.get_axon_ntff_profile_hook = lambda: _state["hook"]
        mod.set_axon_ntff_profile_hook = lambda h: _state.__setitem__("hook", h)
        sys.modules["antenv.axon_hooks"] = mod
        antenv.axon_hooks = mod
        return True
    except Exception:
        return False


def _build():
    import concourse.bass as bass  # noqa: F401
    import concourse.mybir as mybir
    import concourse.tile as tile
    from concourse import bacc
    from concourse.masks import make_identity

    f32 = mybir.dt.float32
    f32r = mybir.dt.float32r
    bf16 = mybir.dt.bfloat16
    EXP = mybir.ActivationFunctionType.Exp

    nc = bacc.Bacc("TRN2", target_bir_lowering=False, debug=False,
                   num_devices=NCORES)

    xT = nc.dram_tensor("xT", [128, KSUB, B * S], bf16, kind="ExternalInput")
    wqkv = nc.dram_tensor("wqkv", [128, KSUB, 3 * HPC * HD], bf16,
                          kind="ExternalInput")
    wout = nc.dram_tensor("wout", [128, KSUB, D], bf16, kind="ExternalInput")
    cosg = nc.dram_tensor("cosg", [128, S], f32, kind="ExternalInput")
    sing = nc.dram_tensor("sing", [128, S], f32, kind="ExternalInput")
    mneg = nc.dram_tensor("mneg", [128, 128], bf16, kind="ExternalInput")
    y = nc.dram_tensor("y", [B, SCW, D], f32, kind="ExternalOutput")

    NQC = S // SC          # qkv s-chunks per batch
    NKT = S // 128         # 16 key tiles
    VOFF = 2 * HPC * HD    # v block column offset in w_sb (512)

    with tile.TileContext(nc) as tc:
        with tc.tile_pool(name="const", bufs=1) as cp, \
             tc.tile_pool(name="stage", bufs=1) as stp, \
             tc.tile_pool(name="dram", bufs=1, space="DRAM") as dp, \
             tc.tile_pool(name="psA", bufs=3, space="PSUM") as psA, \
             tc.tile_pool(name="psOut", bufs=1, space="PSUM") as psO, \
             tc.tile_pool(name="psDen", bufs=1, space="PSUM") as psD, \
             tc.tile_pool(name="w", bufs=1) as wp, \
             tc.tile_pool(name="xc", bufs=2) as xp, \
             tc.tile_pool(name="qkv", bufs=1) as qp, \
             tc.tile_pool(name="rotp", bufs=1) as rp, \
             tc.tile_pool(name="small", bufs=8) as ep:

            cos_sb = cp.tile([128, S], f32, name="cos_sb")
            sin_sb = cp.tile([128, S], f32, name="sin_sb")
            mneg_sb = cp.tile([128, 128], bf16, name="mneg_sb")
            identB = cp.tile([128, 128], bf16, name="identB")
            onesr = cp.tile([1, 128], f32r, name="onesr")
            # matq4[:, j]: [128,128] ones-column at col 32j -> the denominator
            # partial for chunk j lands in (32-aligned) partition 32j of dps
            matq4 = cp.tile([128, 4, 128], bf16, name="matq4")

            # compute-engine constants first (no DMA deps).  f32r/bf16 tiles
            # can't be memset directly (codegen rejects the value type), so
            # build each in an f32 scratch (transient expT slots) + cast-copy.
            ident0 = ep.tile([128, 128], f32, tag="expT", name="ident0")
            make_identity(nc, ident0[:])
            nc.vector.tensor_copy(identB[:], ident0[:])
            ones0 = ep.tile([1, 128], f32, tag="expT", name="ones0")
            nc.vector.memset(ones0[:], 1.0)
            nc.vector.tensor_copy(onesr[:], ones0[:])
            matq0 = ep.tile([128, 4, 128], f32, tag="expT", name="matq0")
            nc.vector.memset(matq0[:], 0.0)
            for j in range(4):
                nc.vector.memset(matq0[:, j, 32 * j:32 * j + 1], 1.0)
            nc.vector.tensor_copy(matq4[:], matq0[:])

            # rope/mask constants ride the (otherwise idle) scalar queue
            nc.scalar.dma_start(cos_sb[:], cosg.ap())
            nc.scalar.dma_start(sin_sb[:], sing.ap())
            nc.scalar.dma_start(mneg_sb[:], mneg.ap())

            # first x chunk: k-split on sync so matmuls can chase the DMA
            xc0 = xp.tile([128, KSUB, SC], bf16, tag="xc", name="xc")
            for kq in range(4):
                nc.sync.dma_start(xc0[:, 4 * kq:4 * kq + 4],
                                  xT.ap()[:, 4 * kq:4 * kq + 4, 0:SC])

            # qk weights on gpsimd in first-use order; the first one k-split
            # so the m2 group's k-loop can chase its DMA
            wqk_t = [None] * (2 * HPC)
            for m in MORDER:
                wt = wp.tile([128, KSUB, 128], bf16, tag=f"w{m}", name=f"w{m}")
                if m == MORDER[0]:
                    for kq in range(4):
                        nc.gpsimd.dma_start(
                            wt[:, 4 * kq:4 * kq + 4],
                            wqkv.ap()[:, 4 * kq:4 * kq + 4,
                                      m * 128:(m + 1) * 128])
                else:
                    nc.gpsimd.dma_start(wt[:],
                                        wqkv.ap()[:, :, m * 128:(m + 1) * 128])
                wqk_t[m] = wt
            wv_t = wp.tile([128, KSUB, HPC * HD], bf16, tag="wv", name="wv")
            nc.sync.dma_start(wv_t[:], wqkv.ap()[:, :, VOFF:VOFF + HPC * HD])

            ibs = {(b, h): dp.tile([NCORES, 128, SCW], bf16, name=f"ib{b}{h}")
                   for b in range(B) for h in range(HPC)}
            obs = {(b, h): dp.tile([NCORES, 128, SCW], bf16, name=f"ob{b}{h}")
                   for b in range(B) for h in range(HPC)}

            def qkv_rope(b, first=False, fin=None):
                qkT = qp.tile([128, 2 * HPC, S], f32r, tag="qkT")
                Vn = qp.tile([128, NKT, HPC * HD], bf16, tag="Vn")

                def rope(m, sc):
                    # fused halves (sin grid pre-swapped):
                    # rt[0:64] = t[64:128]*(-sin); rt[64:128] = t[0:64]*sin;
                    # t *= cos; t += rt
                    lo, hi = sc * SC, (sc + 1) * SC
                    rt = rp.tile([128, S], f32, tag="rot", name="rt")
                    nc.vector.tensor_mul(rt[0:64, lo:hi],
                                         qkT[64:128, m, lo:hi].bitcast(f32),
                                         sin_sb[64:128, lo:hi])
                    nc.vector.tensor_mul(rt[64:128, lo:hi],
                                         qkT[0:64, m, lo:hi].bitcast(f32),
                                         sin_sb[0:64, lo:hi])
                    nc.vector.tensor_mul(qkT[:, m, lo:hi], qkT[:, m, lo:hi],
                                         cos_sb[:, lo:hi])
                    nc.vector.tensor_add(qkT[:, m, lo:hi], qkT[:, m, lo:hi],
                                         rt[:, lo:hi])

                fin = list(fin) if fin else []
                for sc in range(NQC):
                    if first and sc == 0:
                        xc = xc0
                    else:
                        xc = xp.tile([128, KSUB, SC], bf16, tag="xc",
                                     name="xc")
                        off = b * S + sc * SC
                        nc.sync.dma_start(xc[:], xT.ap()[:, :, off:off + SC])
                    for gi, m in enumerate(MORDER):
                        ps = psA.tile([128, 512], f32, tag="bank")
                        for k in range(KSUB):
                            nc.tensor.matmul(
                                ps[:, :SC],
                                wqk_t[m][:, k],
                                xc[:, k],
                                start=(k == 0), stop=(k == KSUB - 1))
                        nc.vector.tensor_copy(
                            qkT[:, m, sc * SC:(sc + 1) * SC], ps[:, :SC])
                        if sc == 0 and fin:
                            # previous pair's finalize pieces, one or two per
                            # m-group; the last slot drains (AllToAll)
                            fin.pop(0)()
                            if gi in (1, 2) and fin and len(fin) > 1:
                                fin.pop(0)()
                            if gi == 3:
                                while fin:
                                    fin.pop(0)()
                        if sc > 0:
                            rope(m, sc)
                    for st2 in range(SC // 128):
                        ps = psA.tile([128, 512], f32, tag="bank")
                        for k in range(KSUB):
                            nc.tensor.matmul(
                                ps[:, :HPC * HD],
                                xc[:, k, st2 * 128:(st2 + 1) * 128],
                                wv_t[:, k],
                                start=(k == 0), stop=(k == KSUB - 1))
                        nc.vector.tensor_copy(
                            Vn[:, sc * (SC // 128) + st2], ps[:, :HPC * HD])
                    if sc == 1:
                        # chunk-0 rope deferred here so the cos/sin DMA has
                        # ample time to land without stalling the DVE queue
                        for m in MORDER:
                            rope(m, 0)
                return qkT, Vn

            def attention(b, h, qkT, Vn, fin_prev=None):
                outT = psO.tile([128, S], f32, tag="outT")
                dps = psD.tile([128, 512], f32, tag="dps")
                st = rp.tile([128, S], bf16, tag="rot", name="st")

                def emit_av_denom(kt, off, ets):
                    q0 = 512 * (kt // 4)
                    for c in range(len(ets)):
                        qs = q0 + c * 512
                        o = off if c == 0 else 0
                        j = qs // 512
                        nc.tensor.matmul(
                            outT[:, qs + o:qs + 512],
                            Vn[:, kt, h * 128:(h + 1) * 128],
                            ets[c][:, o:512],
                            start=(kt == 0),
                            stop=(kt == 4 * j + 3))
                        nc.tensor.matmul(
                            dps[:, o:512],
                            matq4[:, j],
                            ets[c][:, o:512],
                            start=(kt == 0 and c == 0),
                            stop=(kt == NKT - 1 and c == len(ets) - 1))

                fin_prev = list(fin_prev) if fin_prev else []
                prev = None
                for kt in range(NKT):
                    q0 = 512 * (kt // 4)
                    off = 128 * (kt % 4)   # causal start within chunk 0
                    nch = (S - q0) // 512
                    sps = []
                    for c in range(nch):
                        sp = psA.tile([128, 512], f32, tag="bank")
                        sps.append(sp)
                        if c == 0:
                            # -1e9 upper-tri mask for the diagonal 128 block
                            # (bf16: 1 cyc/row at N=128).  The wider scores
                            # MM then accumulates where this wrote and
                            # clean-overwrites the still-pending remainder.
                            nc.tensor.matmul(sp[:, off:off + 128], identB[:],
                                             mneg_sb[:],
                                             start=True, stop=False)
                    for c in range(nch):
                        qs = q0 + c * 512
                        o = off if c == 0 else 0
                        nc.tensor.matmul(
                            sps[c][:, o:512],
                            qkT[:, HPC + h, kt * 128:(kt + 1) * 128],
                            qkT[:, h, qs + o:qs + 512],
                            start=(c != 0), stop=True)
                    if prev is not None:
                        if fin_prev:
                            # one finalize piece of the previous pair per kt,
                            # hidden under this kt's PE work
                            fin_prev.pop(0)()
                        emit_av_denom(*prev)
                    ets = []
                    for c in range(nch):
                        o = off if c == 0 else 0
                        et = ep.tile([128, 512], bf16, tag="expT")
                        ets.append(et)
                        nc.scalar.activation(et[:, o:512], sps[c][:, o:512],
                                             EXP, scale=SCALE)
                    prev = (kt, off, ets)
                emit_av_denom(*prev)

                # Finalize as 6 pieces emitted into the NEXT phase's stream.
                # Piece 0 snapshots everything the next pair will overwrite
                # (dps rows, outT) and MUST run before its kt==1 AV/denom
                # matmuls; pieces 1-4 normalize+ship one chunk each; piece 5
                # triggers the AllToAll.
                state = {}

                def fin_pre():
                    srins = []
                    for j in range(4):
                        # reciprocal_approx ucode ignores the input partition
                        # base: bounce row 32j through a partition-0 tile
                        si = stp.tile([1, 512], f32, tag="srin", bufs=4,
                                      name="si")
                        nc.vector.tensor_copy(si[:],
                                              dps[32 * j:32 * j + 1, :])
                        srins.append(si)
                        sl = slice(j * 512, (j + 1) * 512)
                        nc.vector.tensor_copy(st[:, sl], outT[:, sl])
                    state["srins"] = srins

                def fin_j(j):
                    def go():
                        srow = stp.tile([1, 512], f32, tag="srow")
                        nc.vector.reciprocal_approx_fast(
                            srow[:], state["srins"][j][:])
                        srr = stp.tile([1, 512], f32r, tag="srr")
                        nc.vector.tensor_copy(srr[:], srow[:])
                        bp = psA.tile([128, 512], f32, tag="bank")
                        nc.tensor.matmul(bp[:], onesr[:], srr[:],
                                         start=True, stop=True)
                        sl = slice(j * 512, (j + 1) * 512)
                        nc.vector.tensor_mul(st[:, sl], st[:, sl], bp[:])
                        for jj in (2 * j, 2 * j + 1):
                            nc.sync.dma_start(ibs[(b, h)][jj],
                                              st[:, jj * SCW:(jj + 1) * SCW])
                    return go

                return [fin_pre] + [fin_j(j) for j in range(4)] + \
                    [lambda: a2a(b, h)]

            def load_lhs(b, pool, tag):
                # k-subtile order hh*8+i <-> global head 2i+hh (wout is
                # permuted host-side to match)
                lhs = pool.tile([128, KSUB, SCW], bf16, tag=tag,
                                name=f"lhs{b}")
                for hh in range(HPC):
                    nc.gpsimd.dma_start(
                        lhs[:, hh * NCORES:(hh + 1) * NCORES, :],
                        obs[(b, hh)][:].rearrange("i p s -> p i s"))
                return lhs

            def a2a(b, h):
                nc.gpsimd.collective_compute(
                    "AllToAll", mybir.AluOpType.bypass,
                    replica_groups=[list(range(NCORES))],
                    ins=[ibs[(b, h)].opt()], outs=[obs[(b, h)].opt()])

            # batch 0 compute; its A2A runs while batch 1 computes.
            qkT, Vn = qkv_rope(0, first=True)
            fin00 = attention(0, 0, qkT, Vn)
            fin01 = attention(0, 1, qkT, Vn, fin_prev=fin00)
            qkT, Vn = qkv_rope(1, fin=fin01)

            # w_out chunks (bf16): two 32KB tiles hold all 2048 cols,
            # loaded into the free xc slots during batch-1 attention.
            wo01 = xp.tile([128, KSUB, 1024], bf16, tag="xc", name="wo01")
            nc.scalar.dma_start(wo01[:], wout.ap()[:, :, 0:1024])
            wo23 = xp.tile([128, KSUB, 1024], bf16, tag="xc", name="wo23")
            nc.scalar.dma_start(wo23[:], wout.ap()[:, :, 1024:2048])

            def wo_slice(n):
                t = wo01 if n < 2 else wo23
                return t[:, :, (n % 2) * 512:(n % 2) * 512 + 512]

            fin10 = attention(1, 0, qkT, Vn)
            fin11 = attention(1, 1, qkT, Vn, fin_prev=fin10)

            lhs0 = load_lhs(0, wp, "wv")
            for piece in fin11:
                piece()
            lhs1 = load_lhs(1, qp, "Vn")

            def outproj(b, lhs):
                for n in range(4):
                    wo = wo_slice(n)
                    for m in range(SCW // 128):
                        ps = psA.tile([128, 512], f32, tag="bank")
                        for k in range(KSUB):
                            nc.tensor.matmul(
                                ps[:],
                                lhs[:, k, m * 128:(m + 1) * 128],
                                wo[:, k],
                                start=(k == 0), stop=(k == KSUB - 1))
                        ys = ep.tile([128, 512], f32, tag="expT", name="ys")
                        nc.vector.tensor_copy(ys[:], ps[:])
                        nc.sync.dma_start(
                            y.ap()[b, m * 128:(m + 1) * 128,
                                   n * 512:(n + 1) * 512],
                            ys[:])

            outproj(0, lhs0)   # runs while a2a(1,1) completes
            outproj(1, lhs1)

    nc.finalize()
    return nc


def _host_inputs(x, w_qkv, w_out):
    xTr = np.ascontiguousarray(
        x.reshape(B * S, D).T.reshape(KSUB, 128, B * S).transpose(1, 0, 2)
    ).astype(ml_dtypes.bfloat16)
    horder = [2 * i + hh for hh in range(HPC) for i in range(NCORES)]
    woutr = np.ascontiguousarray(
        w_out.reshape(H, HD, D)[horder].transpose(1, 0, 2)
    ).astype(ml_dtypes.bfloat16)

    half = HD // 2
    inv = (1.0 / (ROPE_BASE ** (np.arange(half, dtype=np.float32) / half))
           ).astype(np.float32)
    ang = (np.arange(S, dtype=np.float32)[:, None] * inv[None, :])  # [S, 64]
    c = np.cos(ang).astype(np.float32).T      # [64, S]
    s = np.sin(ang).astype(np.float32).T
    cosg = np.ascontiguousarray(np.concatenate([c, c], axis=0))
    # pre-swapped: rows 0:64 = +sin (consumed against t[0:64] -> rt[64:128]),
    # rows 64:128 = -sin (consumed against t[64:128] -> rt[0:64])
    sing = np.ascontiguousarray(np.concatenate([s, -s], axis=0))

    # mneg[p, j] = 0 where j >= p else -1e9 (upper-tri of the diagonal
    # 128-block).
    u = np.arange(128)[None, :]
    p = np.arange(128)[:, None]
    mneg = np.where(u >= p, 0.0, -1e9).astype(ml_dtypes.bfloat16)

    maps = []
    for i in range(NCORES):
        h0, h1 = 2 * i, 2 * i + 1
        blocks = []
        for base in (0, D, 2 * D):
            blocks.append(w_qkv[:, base + 128 * h0:base + 128 * (h0 + 1)])
            blocks.append(w_qkv[:, base + 128 * h1:base + 128 * (h1 + 1)])
        shard = np.concatenate(blocks, axis=1)  # [D, 768]
        shard = np.ascontiguousarray(
            shard.reshape(KSUB, 128, 3 * HPC * HD).transpose(1, 0, 2)
        ).astype(ml_dtypes.bfloat16)
        maps.append({"xT": xTr, "wqkv": shard, "wout": woutr,
                     "cosg": cosg, "sing": sing, "mneg": mneg})
    return maps


def kernel(x, w_qkv, w_out):
    from concourse.bass_utils import run_bass_kernel_spmd

    x = np.asarray(x, dtype=np.float32)
    w_qkv = np.asarray(w_qkv, dtype=np.float32)
    w_out = np.asarray(w_out, dtype=np.float32)

    if "nc" not in _CACHE:
        _CACHE["nc"] = _build()
    nc = _CACHE["nc"]

    trace = bool(int(os.environ.get("KERNEL_TRACE", "0")))
    if trace:
        trace = _install_trace_shim()

    in_maps = _host_inputs(x, w_qkv, w_out)
    res = run_bass_kernel_spmd(nc, in_maps, core_ids=list(range(NCORES)),
                               trace=trace)
    _CACHE["last_result"] = res
    # y per core i: [B, 256, D] = output rows [b*2048 + i*256, +256)
    full = np.empty((B * S, D), dtype=np.float32)
    for i in range(NCORES):
        yi = res.results[i]["y"]
        for b in range(B):
            full[b * S + i * SCW: b * S + (i + 1) * SCW] = yi[b]
    return full.reshape(B, S, D)


# revision 38
# speedup vs baseline: 1.0133x; 1.0133x over previous
"""Trainium2 Bass kernel for causal multi-head attention with RoPE.

Problem: x[2,2048,2048] -> qkv proj -> RoPE(q,k) -> causal softmax attention
(16 heads, hd=128) -> out proj.  Sharding: tensor-parallel over heads
(2 heads/core x 8 cores); the output projection contraction is restored
with one AllToAll per batch (head-shards -> sequence-shards), overlapped
with the other batch's compute, so each core computes a disjoint
[2, 256, 2048] slice of the final output.

Schedule highlights:
- startup DMAs split across sync/gpsimd/scalar queues in first-use order
  (k-split x chunk + per-head weights) so the PE starts ~25us earlier.
- RoPE applied per (head-block, s-chunk) right after each PSUM->SBUF
  copy, k-heads first, so attention never waits on a bulk RoPE pass.
- softmax denominators accumulated on the PE itself: per (key-tile,
  chunk) a [128,4] ones-column matmul adds exp-tile partial sums into
  row j of a dedicated PSUM bank (no vector/gpsimd accumulate chain).
- attention out + w_out are cast to bf16: AllToAll payload halves, and
  all four w_out column chunks fit in two resident 32KB SBUF slots, so
  batch-0 out-proj runs contiguously while the last AllToAll completes,
  then batch-1 with zero refetch.
"""

import os
import sys

if "/opt/trn_rl_repo" not in sys.path:
    sys.path.insert(0, "/opt/trn_rl_repo")

import numpy as np
import ml_dtypes

B, S, D = 2, 2048, 2048
H, HD = 16, 128
NCORES = 8
HPC = H // NCORES          # heads per core (2)
ROPE_BASE = 10000.0
SCALE = 1.0 / float(np.sqrt(HD))
SC = 512                   # QKV matmul free-dim chunk (s positions)
KSUB = D // 128            # 16 contraction subtiles
SCW = S // NCORES          # 256: per-core output cols per batch
MORDER = (2, 0, 3, 1)      # k-head blocks first (attention needs them first)

_CACHE = {}


def _install_trace_shim():
    """Optionally register the axon NTFF profile hook (for test.py tracing)."""
    try:
        import types

        if "antenv.axon_hooks" in sys.modules:
            return True
        import antenv
        from trn_agent_boot.trn_boot import _ntff_profile_via_ctypes

        hook = _ntff_profile_via_ctypes("/opt/axon/libaxon_pjrt.so")
        mod = types.ModuleType("antenv.axon_hooks")
        _state = {"hook": hook}
        mod.get_axon_ntff_profile_hook = lambda: _state["hook"]
        mod.set_axon_ntff_profile_hook = lambda h: _state.__setitem__("hook", h)
        sys.modules["antenv.axon_hooks"] = mod
        antenv.axon_hooks = mod
        return True
    except Exception:
        return False


def _build():
    import concourse.bass as bass  # noqa: F401
    import concourse.mybir as mybir
    import concourse.tile as tile
    from concourse import bacc
    from concourse.masks import make_identity

    f32 = mybir.dt.float32
    f32r = mybir.dt.float32r
    bf16 = mybir.dt.bfloat16
    EXP = mybir.ActivationFunctionType.Exp

    nc = bacc.Bacc("TRN2", target_bir_lowering=False, debug=False,
                   num_devices=NCORES)

    xT = nc.dram_tensor("xT", [128, KSUB, B * S], bf16, kind="ExternalInput")
    wqkv = nc.dram_tensor("wqkv", [128, KSUB, 3 * HPC * HD], bf16,
                          kind="ExternalInput")
    wout = nc.dram_tensor("wout", [128, KSUB, D], bf16, kind="ExternalInput")
    cosg = nc.dram_tensor("cosg", [128, S], f32, kind="ExternalInput")
    sing = nc.dram_tensor("sing", [128, S], f32, kind="ExternalInput")
    mneg = nc.dram_tensor("mneg", [128, 128], bf16, kind="ExternalInput")
    y = nc.dram_tensor("y", [B, SCW, D], f32, kind="ExternalOutput")

    NQC = S // SC          # qkv s-chunks per batch
    NKT = S // 128         # 16 key tiles
    VOFF = 2 * HPC * HD    # v block column offset in w_sb (512)

    with tile.TileContext(nc) as tc:
        with tc.tile_pool(name="const", bufs=1) as cp, \
             tc.tile_pool(name="stage", bufs=1) as stp, \
             tc.tile_pool(name="dram", bufs=1, space="DRAM") as dp, \
             tc.tile_pool(name="psA", bufs=3, space="PSUM") as psA, \
             tc.tile_pool(name="psOut", bufs=1, space="PSUM") as psO, \
             tc.tile_pool(name="psDen", bufs=1, space="PSUM") as psD, \
             tc.tile_pool(name="w", bufs=1) as wp, \
             tc.tile_pool(name="xc", bufs=2) as xp, \
             tc.tile_pool(name="qkv", bufs=1) as qp, \
             tc.tile_pool(name="rotp", bufs=1) as rp, \
             tc.tile_pool(name="small", bufs=8) as ep:

            cos_sb = cp.tile([128, S], f32, name="cos_sb")
            sin_sb = cp.tile([128, S], f32, name="sin_sb")
            mneg_sb = cp.tile([128, 128], bf16, name="mneg_sb")
            identB = cp.tile([128, 128], bf16, name="identB")
            onesr = cp.tile([1, 128], bf16, name="onesr")
            # matq4[:, j]: [128,128] ones-column at col 32j -> the denominator
            # partial for chunk j lands in (32-aligned) partition 32j of dps
            matq4 = cp.tile([128, 4, 128], bf16, name="matq4")

            # compute-engine constants first (no DMA deps).  f32r/bf16 tiles
            # can't be memset directly (codegen rejects the value type), so
            # build each in an f32 scratch (transient expT slots) + cast-copy.
            ident0 = ep.tile([128, 128], f32, tag="expT", name="ident0")
            make_identity(nc, ident0[:])
            nc.vector.tensor_copy(identB[:], ident0[:])
            ones0 = ep.tile([1, 128], f32, tag="expT", name="ones0")
            nc.vector.memset(ones0[:], 1.0)
            nc.vector.tensor_copy(onesr[:], ones0[:])
            matq0 = ep.tile([128, 4, 128], f32, tag="expT", name="matq0")
            nc.vector.memset(matq0[:], 0.0)
            for j in range(4):
                nc.vector.memset(matq0[:, j, 32 * j:32 * j + 1], 1.0)
            nc.vector.tensor_copy(matq4[:], matq0[:])

            # rope/mask constants ride the (otherwise idle) scalar queue
            nc.scalar.dma_start(cos_sb[:], cosg.ap())
            nc.scalar.dma_start(sin_sb[:], sing.ap())
            nc.scalar.dma_start(mneg_sb[:], mneg.ap())

            # first x chunk: k-split on sync so matmuls can chase the DMA
            xc0 = xp.tile([128, KSUB, SC], bf16, tag="xc", name="xc")
            for kq in range(4):
                nc.sync.dma_start(xc0[:, 4 * kq:4 * kq + 4],
                                  xT.ap()[:, 4 * kq:4 * kq + 4, 0:SC])

            # qk weights on gpsimd in first-use order; the first one k-split
            # so the m2 group's k-loop can chase its DMA
            wqk_t = [None] * (2 * HPC)
            for m in MORDER:
                wt = wp.tile([128, KSUB, 128], bf16, tag=f"w{m}", name=f"w{m}")
                if m == MORDER[0]:
                    for kq in range(4):
                        nc.gpsimd.dma_start(
                            wt[:, 4 * kq:4 * kq + 4],
                            wqkv.ap()[:, 4 * kq:4 * kq + 4,
                                      m * 128:(m + 1) * 128])
                else:
                    nc.gpsimd.dma_start(wt[:],
                                        wqkv.ap()[:, :, m * 128:(m + 1) * 128])
                wqk_t[m] = wt
            wv_t = wp.tile([128, KSUB, HPC * HD], bf16, tag="wv", name="wv")
            nc.sync.dma_start(wv_t[:], wqkv.ap()[:, :, VOFF:VOFF + HPC * HD])

            ibs = {(b, h): dp.tile([NCORES, 128, SCW], bf16, name=f"ib{b}{h}")
                   for b in range(B) for h in range(HPC)}
            obs = {(b, h): dp.tile([NCORES, 128, SCW], bf16, name=f"ob{b}{h}")
                   for b in range(B) for h in range(HPC)}

            def qkv_rope(b, first=False, fin=None):
                qkT = qp.tile([128, 2 * HPC, S], f32r, tag="qkT")
                Vn = qp.tile([128, NKT, HPC * HD], bf16, tag="Vn")

                def rope(m, sc):
                    # fused halves (sin grid pre-swapped):
                    # rt[0:64] = t[64:128]*(-sin); rt[64:128] = t[0:64]*sin;
                    # t *= cos; t += rt
                    lo, hi = sc * SC, (sc + 1) * SC
                    rt = rp.tile([128, S], f32, tag="rot", name="rt")
                    nc.vector.tensor_mul(rt[0:64, lo:hi],
                                         qkT[64:128, m, lo:hi].bitcast(f32),
                                         sin_sb[64:128, lo:hi])
                    nc.vector.tensor_mul(rt[64:128, lo:hi],
                                         qkT[0:64, m, lo:hi].bitcast(f32),
                                         sin_sb[0:64, lo:hi])
                    nc.vector.tensor_mul(qkT[:, m, lo:hi], qkT[:, m, lo:hi],
                                         cos_sb[:, lo:hi])
                    nc.vector.tensor_add(qkT[:, m, lo:hi], qkT[:, m, lo:hi],
                                         rt[:, lo:hi])

                fin = list(fin) if fin else []
                for sc in range(NQC):
                    if first and sc == 0:
                        xc = xc0
                    else:
                        xc = xp.tile([128, KSUB, SC], bf16, tag="xc",
                                     name="xc")
                        off = b * S + sc * SC
                        nc.sync.dma_start(xc[:], xT.ap()[:, :, off:off + SC])
                    for gi, m in enumerate(MORDER):
                        ps = psA.tile([128, 512], f32, tag="bank")
                        for k in range(KSUB):
                            nc.tensor.matmul(
                                ps[:, :SC],
                                wqk_t[m][:, k],
                                xc[:, k],
                                start=(k == 0), stop=(k == KSUB - 1))
                        nc.vector.tensor_copy(
                            qkT[:, m, sc * SC:(sc + 1) * SC], ps[:, :SC])
                        if sc == 0 and fin:
                            # previous pair's finalize pieces, one or two per
                            # m-group; the last slot drains (AllToAll)
                            fin.pop(0)()
                            if gi in (1, 2) and fin and len(fin) > 1:
                                fin.pop(0)()
                            if gi == 3:
                                while fin:
                                    fin.pop(0)()
                        if sc > 0:
                            rope(m, sc)
                    for st2 in range(SC // 128):
                        ps = psA.tile([128, 512], f32, tag="bank")
                        for k in range(KSUB):
                            nc.tensor.matmul(
                                ps[:, :HPC * HD],
                                xc[:, k, st2 * 128:(st2 + 1) * 128],
                                wv_t[:, k],
                                start=(k == 0), stop=(k == KSUB - 1))
                        nc.vector.tensor_copy(
                            Vn[:, sc * (SC // 128) + st2], ps[:, :HPC * HD])
                    if sc == 1:
                        # chunk-0 rope deferred here so the cos/sin DMA has
                        # ample time to land without stalling the DVE queue
                        for m in MORDER:
                            rope(m, 0)
                return qkT, Vn

            def attention(b, h, qkT, Vn, fin_prev=None):
                outT = psO.tile([128, S], f32, tag="outT")
                dps = psD.tile([128, 512], f32, tag="dps")
                st = rp.tile([128, S], bf16, tag="rot", name="st")

                def emit_av_denom(kt, off, ets):
                    q0 = 512 * (kt // 4)
                    for c in range(len(ets)):
                        qs = q0 + c * 512
                        o = off if c == 0 else 0
                        j = qs // 512
                        nc.tensor.matmul(
                            outT[:, qs + o:qs + 512],
                            Vn[:, kt, h * 128:(h + 1) * 128],
                            ets[c][:, o:512],
                            start=(kt == 0),
                            stop=(kt == 4 * j + 3))
                        nc.tensor.matmul(
                            dps[:, o:512],
                            matq4[:, j],
                            ets[c][:, o:512],
                            start=(kt == 0 and c == 0),
                            stop=(kt == NKT - 1 and c == len(ets) - 1))

                fin_prev = list(fin_prev) if fin_prev else []
                prev = None
                for kt in range(NKT):
                    q0 = 512 * (kt // 4)
                    off = 128 * (kt % 4)   # causal start within chunk 0
                    nch = (S - q0) // 512
                    sps = []
                    for c in range(nch):
                        sp = psA.tile([128, 512], f32, tag="bank")
                        sps.append(sp)
                        if c == 0:
                            # -1e9 upper-tri mask for the diagonal 128 block
                            # (bf16: 1 cyc/row at N=128).  The wider scores
                            # MM then accumulates where this wrote and
                            # clean-overwrites the still-pending remainder.
                            nc.tensor.matmul(sp[:, off:off + 128], identB[:],
                                             mneg_sb[:],
                                             start=True, stop=False)
                    for c in range(nch):
                        qs = q0 + c * 512
                        o = off if c == 0 else 0
                        nc.tensor.matmul(
                            sps[c][:, o:512],
                            qkT[:, HPC + h, kt * 128:(kt + 1) * 128],
                            qkT[:, h, qs + o:qs + 512],
                            start=(c != 0), stop=True)
                    if prev is not None:
                        if fin_prev:
                            # one finalize piece of the previous pair per kt,
                            # hidden under this kt's PE work
                            fin_prev.pop(0)()
                        emit_av_denom(*prev)
                    ets = []
                    for c in range(nch):
                        o = off if c == 0 else 0
                        et = ep.tile([128, 512], bf16, tag="expT")
                        ets.append(et)
                        nc.scalar.activation(et[:, o:512], sps[c][:, o:512],
                                             EXP, scale=SCALE)
                    prev = (kt, off, ets)
                emit_av_denom(*prev)

                # Finalize as 6 pieces emitted into the NEXT phase's stream.
                # Piece 0 snapshots everything the next pair will overwrite
                # (dps rows, outT) and MUST run before its kt==1 AV/denom
                # matmuls; pieces 1-4 normalize+ship one chunk each; piece 5
                # triggers the AllToAll.
                state = {}

                def fin_pre():
                    srins = []
                    for j in range(4):
                        # reciprocal_approx ucode ignores the input partition
                        # base: bounce row 32j through a partition-0 tile
                        si = stp.tile([1, 512], f32, tag="srin", bufs=4,
                                      name="si")
                        nc.vector.tensor_copy(si[:],
                                              dps[32 * j:32 * j + 1, :])
                        srins.append(si)
                        sl = slice(j * 512, (j + 1) * 512)
                        nc.vector.tensor_copy(st[:, sl], outT[:, sl])
                    state["srins"] = srins

                def fin_j(j):
                    def go():
                        srow = stp.tile([1, 512], f32, tag="srow")
                        nc.vector.reciprocal_approx_fast(
                            srow[:], state["srins"][j][:])
                        srr = stp.tile([1, 512], bf16, tag="srr")
                        nc.vector.tensor_copy(srr[:], srow[:])
                        bp = psA.tile([128, 512], f32, tag="bank")
                        nc.tensor.matmul(bp[:], onesr[:], srr[:],
                                         start=True, stop=True)
                        sl = slice(j * 512, (j + 1) * 512)
                        nc.vector.tensor_mul(st[:, sl], st[:, sl], bp[:])
                        for jj in (2 * j, 2 * j + 1):
                            nc.sync.dma_start(ibs[(b, h)][jj],
                                              st[:, jj * SCW:(jj + 1) * SCW])
                    return go

                return [fin_pre] + [fin_j(j) for j in range(4)] + \
                    [lambda: a2a(b, h)]

            def load_lhs(b, pool, tag):
                # k-subtile order hh*8+i <-> global head 2i+hh (wout is
                # permuted host-side to match)
                lhs = pool.tile([128, KSUB, SCW], bf16, tag=tag,
                                name=f"lhs{b}")
                for hh in range(HPC):
                    nc.gpsimd.dma_start(
                        lhs[:, hh * NCORES:(hh + 1) * NCORES, :],
                        obs[(b, hh)][:].rearrange("i p s -> p i s"))
                return lhs

            def a2a(b, h):
                nc.gpsimd.collective_compute(
                    "AllToAll", mybir.AluOpType.bypass,
                    replica_groups=[list(range(NCORES))],
                    ins=[ibs[(b, h)].opt()], outs=[obs[(b, h)].opt()])

            # batch 0 compute; its A2A runs while batch 1 computes.
            qkT, Vn = qkv_rope(0, first=True)
            fin00 = attention(0, 0, qkT, Vn)
            fin01 = attention(0, 1, qkT, Vn, fin_prev=fin00)
            qkT, Vn = qkv_rope(1, fin=fin01)

            # w_out chunks (bf16): two 32KB tiles hold all 2048 cols,
            # loaded into the free xc slots during batch-1 attention.
            wo01 = xp.tile([128, KSUB, 1024], bf16, tag="xc", name="wo01")
            nc.sync.dma_start(wo01[:], wout.ap()[:, :, 0:1024])
            wo23 = xp.tile([128, KSUB, 1024], bf16, tag="xc", name="wo23")
            nc.sync.dma_start(wo23[:], wout.ap()[:, :, 1024:2048])

            def wo_slice(n):
                t = wo01 if n < 2 else wo23
                return t[:, :, (n % 2) * 512:(n % 2) * 512 + 512]

            fin10 = attention(1, 0, qkT, Vn)
            fin11 = attention(1, 1, qkT, Vn, fin_prev=fin10)

            lhs0 = load_lhs(0, wp, "wv")
            for piece in fin11:
                piece()
            lhs1 = load_lhs(1, qp, "Vn")

            def outproj(b, lhs):
                for n in range(4):
                    wo = wo_slice(n)
                    for m in range(SCW // 128):
                        ps = psA.tile([128, 512], f32, tag="bank")
                        for k in range(KSUB):
                            nc.tensor.matmul(
                                ps[:],
                                lhs[:, k, m * 128:(m + 1) * 128],
                                wo[:, k],
                                start=(k == 0), stop=(k == KSUB - 1))
                        ys = ep.tile([128, 512], f32, tag="expT", name="ys")
                        nc.vector.tensor_copy(ys[:], ps[:])
                        nc.sync.dma_start(
                            y.ap()[b, m * 128:(m + 1) * 128,
                                   n * 512:(n + 1) * 512],
                            ys[:])

            outproj(0, lhs0)   # runs while a2a(1,1) completes
            outproj(1, lhs1)

    nc.finalize()
    return nc


def _host_inputs(x, w_qkv, w_out):
    xTr = np.ascontiguousarray(
        x.reshape(B * S, D).T.reshape(KSUB, 128, B * S).transpose(1, 0, 2)
    ).astype(ml_dtypes.bfloat16)
    horder = [2 * i + hh for hh in range(HPC) for i in range(NCORES)]
    woutr = np.ascontiguousarray(
        w_out.reshape(H, HD, D)[horder].transpose(1, 0, 2)
    ).astype(ml_dtypes.bfloat16)

    half = HD // 2
    inv = (1.0 / (ROPE_BASE ** (np.arange(half, dtype=np.float32) / half))
           ).astype(np.float32)
    ang = (np.arange(S, dtype=np.float32)[:, None] * inv[None, :])  # [S, 64]
    c = np.cos(ang).astype(np.float32).T      # [64, S]
    s = np.sin(ang).astype(np.float32).T
    cosg = np.ascontiguousarray(np.concatenate([c, c], axis=0))
    # pre-swapped: rows 0:64 = +sin (consumed against t[0:64] -> rt[64:128]),
    # rows 64:128 = -sin (consumed against t[64:128] -> rt[0:64])
    sing = np.ascontiguousarray(np.concatenate([s, -s], axis=0))

    # mneg[p, j] = 0 where j >= p else -1e9 (upper-tri of the diagonal
    # 128-block).
    u = np.arange(128)[None, :]
    p = np.arange(128)[:, None]
    mneg = np.where(u >= p, 0.0, -1e9).astype(ml_dtypes.bfloat16)

    maps = []
    for i in range(NCORES):
        h0, h1 = 2 * i, 2 * i + 1
        blocks = []
        for base in (0, D, 2 * D):
            blocks.append(w_qkv[:, base + 128 * h0:base + 128 * (h0 + 1)])
            blocks.append(w_qkv[:, base + 128 * h1:base + 128 * (h1 + 1)])
        shard = np.concatenate(blocks, axis=1)  # [D, 768]
        shard = np.ascontiguousarray(
            shard.reshape(KSUB, 128, 3 * HPC * HD).transpose(1, 0, 2)
        ).astype(ml_dtypes.bfloat16)
        maps.append({"xT": xTr, "wqkv": shard, "wout": woutr,
                     "cosg": cosg, "sing": sing, "mneg": mneg})
    return maps


def kernel(x, w_qkv, w_out):
    from concourse.bass_utils import run_bass_kernel_spmd

    x = np.asarray(x, dtype=np.float32)
    w_qkv = np.asarray(w_qkv, dtype=np.float32)
    w_out = np.asarray(w_out, dtype=np.float32)

    if "nc" not in _CACHE:
        _CACHE["nc"] = _build()
    nc = _CACHE["nc"]

    trace = bool(int(os.environ.get("KERNEL_TRACE", "0")))
    if trace:
        trace = _install_trace_shim()

    in_maps = _host_inputs(x, w_qkv, w_out)
    res = run_bass_kernel_spmd(nc, in_maps, core_ids=list(range(NCORES)),
                               trace=trace)
    _CACHE["last_result"] = res
    # y per core i: [B, 256, D] = output rows [b*2048 + i*256, +256)
    full = np.empty((B * S, D), dtype=np.float32)
    for i in range(NCORES):
        yi = res.results[i]["y"]
        for b in range(B):
            full[b * S + i * SCW: b * S + (i + 1) * SCW] = yi[b]
    return full.reshape(B, S, D)
